# revision 17
# baseline (speedup 1.0000x reference)
"""Trainium2 Bass kernel for windowed multi-head attention with relative
position bias (Swin-style block):

    qkv = x @ qkv_w.T + [q_bias, 0, v_bias]
    q, k, v = split(qkv);  q *= hd**-0.5
    attn = softmax(q @ k.T + rel_table[rel_index])
    out  = (attn @ v) @ proj_w.T + proj_b

Shapes: x [8, 32, 32, 768], 12 heads, head_dim 64, N=1024 tokens.

Sharding: pure data-parallel - one batch element per NeuronCore, 8 cores,
no collectives. Each core runs an identical NEFF on its own slice.

v3 dataflow (ACT-exp is the engine floor; pipeline everything under it):
  - all inputs fp16 (host-converted); all matmuls fp16 (norm broadcast f32r).
  - per head-pair j, attention starts as soon as its Q/K tiles land: QKV
    production (PE) is interleaved with attention so the scalar engine (exp)
    saturates from ~t=14us and the PE never idles past the HAM window.
  - S^T tiles [128k, 1024q] as ROW-PACKED pairs of K=64 matmuls (head 2j on
    array rows 0-63, head 2j+1 on rows 64-127, concurrent).
  - P^T = exp(S^T) on ACT (psum->sbuf fp16), multiplied by host-precomputed
    exp(bias^T) on DVE (fp16 2x).
  - AV with a 65-column stationary [V_h | 1]: unnormalized out2^T plus the
    softmax row sums in psum row 64.
  - normalization fully on-chip and delayed by one pair so its chain (ACT
    row copy -> DVE approx reciprocal -> K=1 selector matmuls -> DVE apply)
    overlaps the next pair's matmuls instead of stalling the psum ring.
  - proj y^T = wproj^T @ attn at the end; y shipped fp16, host casts.
All PSUM flows through tags pss (2x2 banks) + po (2x2 banks) = 8 banks;
QK/V/pinv tiles ride the pss ring.
"""

import numpy as np

_CACHE = {}

B = 8
WS = 32
N = WS * WS            # 1024 tokens
C = 768
NH = 12
HD = 64
P = 128
QC = 2                 # q chunks of 512
QN = N // QC           # 512
KT = N // P            # 8 k tiles
CT = C // P            # 6 contraction tiles
NPAIR = NH // 2        # 6 head pairs
OT_QK = 2 * CT         # 12 output tiles for q,k rows
VC = 2                 # v output chunks of 384
VN = C // VC           # 384


def _build():
    import concourse.bass as bass
    import concourse.bacc as bacc
    import concourse.mybir as mybir
    import concourse.tile as tile

    f32 = mybir.dt.float32
    f32r = mybir.dt.float32r
    f16 = mybir.dt.float16
    AF = mybir.ActivationFunctionType

    nc = bacc.Bacc(None, target_bir_lowering=False)

    xT_d = nc.dram_tensor("xT", [C, N], f16, kind="ExternalInput")
    # per-pair slabs: [jp][c][0:128]=wq tile (scaled), [128:256]=wk tile
    wqk_d = nc.dram_tensor("wqk", [NPAIR, C, 2 * P], f16, kind="ExternalInput")
    wv_d = nc.dram_tensor("wv", [C, C], f16, kind="ExternalInput")
    wproj_d = nc.dram_tensor("wproj", [C, C], f16, kind="ExternalInput")
    cn_d = nc.dram_tensor("cn", [P, 512], f32, kind="ExternalInput")
    vb_d = nc.dram_tensor("vb", [C], f32, kind="ExternalInput")
    expBT_d = nc.dram_tensor("expBT", [NH, KT, P, N], f16, kind="ExternalInput")
    yT_d = nc.dram_tensor("yT", [C, N], f16, kind="ExternalOutput")

    with tile.TileContext(nc) as tc:
        with (
            tc.tile_pool(name="cst", bufs=1) as cst,
            tc.tile_pool(name="bias_pool", bufs=4) as bias_pool,
            tc.tile_pool(name="pt_pool", bufs=4) as pt_pool,
        ):
            # ---- persistent SBUF ----
            xT_s = cst.tile([P, CT, N], f16)
            wqk_s = cst.tile([P, NPAIR, CT, 2 * P], f16)
            wv_s = cst.tile([P, CT, C], f16)
            wproj_s = cst.tile([P, CT, C], f16)
            cn_s = cst.tile([P, 512], f32)
            q_t = cst.tile([P, NPAIR, N], f16)
            k_t = cst.tile([P, NPAIR, N], f16)
            v_aug = cst.tile([P, KT, NH, HD + 1], f16)
            attn = cst.tile([P, NPAIR, N], f16)
            vb_bc = cst.tile([P, C], f32)
            # selector ones-rows at partitions 0 and 32 (engine accesses must
            # start at 32-aligned partitions, so the two 1/sums rows live at
            # rows 0 and 32 of 33-row tiles)
            sel_s = cst.tile([33, P], f16)
            srows = cst.tile([33, N], f32)
            srinv = cst.tile([33, N], f32)
            s2f16 = cst.tile([33, N], f16)
            warm = cst.tile([1, 16], f32)

            biasb = {}

            def load_bias(h):
                biasb[h] = bias_pool.tile([P, KT, N], f16, tag="biasT",
                                          name=f"biasT{h}")
                nc.sync.dma_start(
                    biasb[h],
                    expBT_d[h].rearrange("kt p q -> p kt q"))

            # ---- DMAs: phase-1-critical first; bias after (it otherwise
            # starves the startup-critical transfers for bandwidth) ----
            xT_src = xT_d[:].rearrange("(k p) t -> p k t", p=P)
            wv_src = wv_d[:].rearrange("(k p) o -> p k o", p=P)
            wproj_src = wproj_d[:].rearrange("(k p) o -> p k o", p=P)
            wqk_src = wqk_d[:].rearrange("jp (k p) o -> p jp k o", p=P)

            nc.sync.dma_start(xT_s[:, 0:3, :], xT_src[:, 0:3, :])
            nc.sync.dma_start(xT_s[:, 3:6, :], xT_src[:, 3:6, :])
            nc.sync.dma_start(wqk_s[:, 0, :, :], wqk_src[:, 0, :, :])
            nc.sync.dma_start(wv_s, wv_src)
            load_bias(0)
            load_bias(1)
            nc.sync.dma_start(cn_s, cn_d[:])
            nc.sync.dma_start(
                vb_bc, bass.AP(tensor=vb_d, offset=0, ap=[[0, P], [1, C]]))
            nc.sync.dma_start(wqk_s[:, 1:NPAIR, :, :],
                              wqk_src[:, 1:NPAIR, :, :])
            qkb_s = cn_s[:, 0:OT_QK]
            pb_s = cn_s[:, 16:16 + CT]

            nc.vector.memset(v_aug[:, :, :, HD:HD + 1], 1.0)
            nc.vector.memset(sel_s[0:1, 0:64], 1.0)
            nc.vector.memset(sel_s[32:33, 0:64], 1.0)
            nc.vector.memset(srinv, 1.0)
            nc.vector.memset(srows, 1.0)
            nc.vector.memset(warm, 0.0)
            # pull the exp table set in during the initial DMA window
            nc.scalar.activation(warm, warm, AF.Exp, bias=0.0, scale=1.0)

            # ---- main PSUM pool: everything rides two tag rings ----
            with tc.tile_pool(name="ps", bufs=1, space="PSUM") as ps:

                def qk_tile(jp, kside):
                    """Produce q_t/k_t tile jp: [128 o, 1024 t]."""
                    o0 = P if kside else 0
                    jj = (CT + jp) if kside else jp
                    for qc in range(QC):
                        pq = ps.tile([P, QN], f32, tag="pss", bufs=4,
                                     name=f"pq{jp}_{kside}_{qc}")
                        for k in range(CT):
                            nc.tensor.matmul(
                                pq,
                                wqk_s[:, jp, k, o0:o0 + P],
                                xT_s[:, k, qc * QN:(qc + 1) * QN],
                                start=(k == 0), stop=(k == CT - 1))
                        dst = k_t if kside else q_t
                        nc.vector.tensor_scalar_add(
                            dst[:, jp, qc * QN:(qc + 1) * QN], pq,
                            qkb_s[:, jj:jj + 1])

                def v_tile(tt):
                    """Produce v_aug rows for t-tile tt (all 12 heads)."""
                    for vc in range(VC):
                        pv = ps.tile([P, QN], f32, tag="pss", bufs=4,
                                     name=f"pv{tt}_{vc}")
                        for k in range(CT):
                            nc.tensor.matmul(
                                pv[:, 0:VN],
                                xT_s[:, k, tt * P:(tt + 1) * P],
                                wv_s[:, k, vc * VN:(vc + 1) * VN],
                                start=(k == 0), stop=(k == CT - 1))
                        h0 = vc * (NH // VC)
                        nc.vector.tensor_add(
                            v_aug[:, tt, h0:h0 + NH // VC, 0:HD],
                            pv[:, 0:VN],
                            vb_bc[:, vc * VN:(vc + 1) * VN])

                pos = {}

                def norm_pair(jp):
                    """sums -> 1/sums -> broadcast -> evict + apply pair jp."""
                    po_a, po_b = pos.pop(jp)
                    nc.scalar.activation(srows[0:1, :], po_a[64:65, :],
                                         AF.Identity, bias=0.0, scale=1.0)
                    nc.scalar.activation(srows[32:33, :], po_b[64:65, :],
                                         AF.Identity, bias=0.0, scale=1.0)
                    nc.vector.reciprocal_approx_fast(srinv, srows)
                    with nc.allow_low_precision(reason="1/sums in f16"):
                        nc.vector.tensor_copy(s2f16, srinv)
                    nc.vector.tensor_copy(attn[0:64, jp, :], po_a[0:64, :])
                    nc.vector.tensor_copy(attn[64:128, jp, :], po_b[0:64, :])
                    for qc in range(QC):
                        qs = slice(qc * QN, (qc + 1) * QN)
                        pinv = ps.tile([P, QN], f32, tag="pss", bufs=4,
                                       name=f"pinv{jp}_{qc}")
                        nc.tensor.matmul(pinv[0:64, :],
                                         sel_s[0:1, 0:64],
                                         s2f16[0:1, qs],
                                         start=True, stop=True)
                        nc.tensor.matmul(pinv[64:128, :],
                                         sel_s[32:33, 0:64],
                                         s2f16[32:33, qs],
                                         start=True, stop=True)
                        nc.vector.tensor_mul(attn[:, jp, qs],
                                             attn[:, jp, qs], pinv)

                # phase 1 pre: QK pair 0, then V
                qk_tile(0, 0)
                qk_tile(0, 1)
                for tt in range(KT):
                    v_tile(tt)

                # ---- head-pair loop ----
                for jp in range(NPAIR):
                    ha, hb = 2 * jp, 2 * jp + 1
                    if jp + 1 < NPAIR:
                        qk_tile(jp + 1, 0)
                        qk_tile(jp + 1, 1)
                    if 2 * jp + 2 < NH:
                        load_bias(2 * jp + 2)
                    if jp == 2:
                        nc.sync.dma_start(wproj_s, wproj_src)

                    po_a = ps.tile([HD + 1, N], f32, tag="po", bufs=2,
                                   name=f"poa{jp}")
                    po_b = ps.tile([HD + 1, N], f32, tag="po", bufs=2,
                                   name=f"pob{jp}")
                    pos[jp] = (po_a, po_b)
                    pt2 = {}

                    def s_block(kt, jp=jp, ha=ha, hb=hb, pt2=pt2):
                        g, r = kt // 2, kt % 2
                        for (h, base) in ((ha, 0), (hb, 64)):
                            if r == 0:
                                pt2[(h, g)] = pt_pool.tile(
                                    [P, 2, N], f16, tag="pt",
                                    name=f"pt{h}_{g}")
                            pt = pt2[(h, g)]
                            for qc in range(QC):
                                pS = ps.tile([P, QN], f32, tag="pss", bufs=4,
                                             name=f"pS{h}_{kt}_{qc}")
                                nc.tensor.matmul(
                                    pS,
                                    k_t[base:base + HD, jp,
                                        kt * P:(kt + 1) * P],
                                    q_t[base:base + HD, jp,
                                        qc * QN:(qc + 1) * QN],
                                    start=True, stop=True)
                                nc.scalar.activation(
                                    pt[:, r, qc * QN:(qc + 1) * QN], pS,
                                    AF.Exp, bias=0.0, scale=1.0)
                            if r == 1:
                                nc.vector.tensor_mul(
                                    pt, pt, biasb[h][:, 2 * g:2 * g + 2, :])

                    def av_block(kt, jp=jp, ha=ha, hb=hb, pt2=pt2,
                                 po_a=po_a, po_b=po_b):
                        g, r = kt // 2, kt % 2
                        for (h, po) in ((ha, po_a), (hb, po_b)):
                            pt = pt2[(h, g)]
                            for qc in range(QC):
                                nc.tensor.matmul(
                                    po[:, qc * QN:(qc + 1) * QN],
                                    v_aug[:, kt, h, :],
                                    pt[:, r, qc * QN:(qc + 1) * QN],
                                    start=(kt == 0), stop=(kt == KT - 1))

                    s_block(0)
                    s_block(1)
                    if jp > 0:
                        norm_pair(jp - 1)
                    av_block(0)
                    for kt in range(2, KT):
                        if kt == 4 and 2 * jp + 3 < NH:
                            load_bias(2 * jp + 3)
                        s_block(kt)
                        av_block(kt - 1)
                    av_block(KT - 1)

                norm_pair(NPAIR - 1)

            # ---- phase 3: y^T = wproj^T @ attn ----
            with (
                tc.tile_pool(name="p3", bufs=1) as p3,
                tc.tile_pool(name="ps_y", bufs=1, space="PSUM") as ps_y,
            ):
                yT_dst = yT_d[:].rearrange("(j p) t -> p j t", p=P)
                for j in range(CT):
                    py = ps_y.tile([P, N], f32, tag="py", bufs=3,
                                   name=f"py{j}")
                    for k in range(CT):
                        for qc in range(QC):
                            nc.tensor.matmul(
                                py[:, qc * QN:(qc + 1) * QN],
                                wproj_s[:, k, j * P:(j + 1) * P],
                                attn[:, k, qc * QN:(qc + 1) * QN],
                                start=(k == 0), stop=(k == CT - 1))
                    yb = p3.tile([P, N], f16, tag="yb", bufs=2, name=f"yb{j}")
                    nc.vector.tensor_scalar_add(yb, py, pb_s[:, j:j + 1])
                    nc.sync.dma_start(yT_dst[:, j, :], yb)

    nc.compile()
    return nc


def _get_nc():
    if "nc" not in _CACHE:
        _CACHE["nc"] = _build()
    return _CACHE["nc"]


def prepare_inputs(x, qkv_w, q_bias, v_bias, proj_w, proj_b, rel_table,
                   rel_index):
    """Host-side resharding/layout prep. Returns per-core input maps."""
    scale = HD ** -0.5
    x = np.asarray(x, np.float32)
    qkv_w = np.asarray(qkv_w, np.float32)
    q_bias = np.asarray(q_bias, np.float32)
    v_bias = np.asarray(v_bias, np.float32)
    proj_w = np.asarray(proj_w, np.float32)
    proj_b = np.asarray(proj_b, np.float32)
    rel_table = np.asarray(rel_table, np.float32)
    rel_index = np.asarray(rel_index)

    wqT = (qkv_w[0:C, :] * scale).T        # [c, o]
    wkT = qkv_w[C:2 * C, :].T
    wv = qkv_w[2 * C:3 * C, :]
    # per-pair slabs [jp, c, 256]: q j-tile then k j-tile
    wqk = np.empty((NPAIR, C, 2 * P), np.float16)
    for jp in range(NPAIR):
        wqk[jp, :, 0:P] = wqT[:, jp * P:(jp + 1) * P]
        wqk[jp, :, P:2 * P] = wkT[:, jp * P:(jp + 1) * P]
    wv_t = np.ascontiguousarray(wv.T.astype(np.float16))         # [c, C]
    wproj = np.ascontiguousarray(proj_w.T.astype(np.float16))    # [c, co]
    qkb = np.concatenate([q_bias * scale, np.zeros(C, np.float32)])
    qkb = qkb.reshape(OT_QK, P).T                                # [P, 12]
    pb = proj_b.reshape(CT, P).T                                 # [P, 6]
    cn = np.zeros((P, 512), np.float32)
    cn[:, 0:OT_QK] = qkb
    cn[:, 16:16 + CT] = pb

    # bias[q, k, h] = rel_table[rel_index[q, k]]; ship exp(biasT[h, k, q])
    # so the kernel folds the softmax bias multiplicatively into P^T
    bias = rel_table[rel_index.reshape(-1)].reshape(N, N, NH)
    expBT = np.exp(bias.transpose(2, 1, 0), dtype=np.float32)
    expBT = np.ascontiguousarray(
        expBT.reshape(NH, KT, P, N).astype(np.float16))

    shared = {
        "wqk": wqk, "wv": wv_t, "wproj": wproj, "cn": cn,
        "vb": v_bias, "expBT": expBT,
    }
    in_maps = []
    for b in range(B):
        xt = np.ascontiguousarray(
            x[b].reshape(N, C).T.astype(np.float16))
        in_maps.append({"xT": xt, **shared})
    return in_maps


def kernel(x, qkv_w, q_bias, v_bias, proj_w, proj_b, rel_table, rel_index,
           _trace=False):
    from concourse.bass_utils import run_bass_kernel_spmd

    nc = _get_nc()
    in_maps = prepare_inputs(x, qkv_w, q_bias, v_bias, proj_w, proj_b,
                             rel_table, rel_index)
    kwargs = {}
    if _trace:
        import concourse.bass_utils as _bu
        _bu.upload_artifacts = lambda tmpdir: tmpdir
        kwargs = {"trace": True}
    res = run_bass_kernel_spmd(nc, in_maps, core_ids=list(range(B)), **kwargs)
    out = np.empty((B, WS, WS, C), np.float32)
    for b in range(B):
        out[b] = res.results[b]["yT"].astype(np.float32).T.reshape(WS, WS, C)
    if _trace:
        _CACHE["last_result"] = res
    return out


# revision 18
# speedup vs baseline: 1.0312x; 1.0312x over previous
"""Trainium2 Bass kernel for windowed multi-head attention with relative
position bias (Swin-style block):

    qkv = x @ qkv_w.T + [q_bias, 0, v_bias]
    q, k, v = split(qkv);  q *= hd**-0.5
    attn = softmax(q @ k.T + rel_table[rel_index])
    out  = (attn @ v) @ proj_w.T + proj_b

Shapes: x [8, 32, 32, 768], 12 heads, head_dim 64, N=1024 tokens.

Sharding: pure data-parallel - one batch element per NeuronCore, 8 cores,
no collectives. Each core runs an identical NEFF on its own slice.

v3 dataflow (ACT-exp is the engine floor; pipeline everything under it):
  - all inputs fp16 (host-converted); all matmuls fp16 (norm broadcast f32r).
  - per head-pair j, attention starts as soon as its Q/K tiles land: QKV
    production (PE) is interleaved with attention so the scalar engine (exp)
    saturates from ~t=14us and the PE never idles past the HAM window.
  - S^T tiles [128k, 1024q] as ROW-PACKED pairs of K=64 matmuls (head 2j on
    array rows 0-63, head 2j+1 on rows 64-127, concurrent).
  - P^T = exp(S^T) on ACT (psum->sbuf fp16), multiplied by host-precomputed
    exp(bias^T) on DVE (fp16 2x).
  - AV with a 65-column stationary [V_h | 1]: unnormalized out2^T plus the
    softmax row sums in psum row 64.
  - normalization fully on-chip and delayed by one pair so its chain (ACT
    row copy -> DVE approx reciprocal -> K=1 selector matmuls -> DVE apply)
    overlaps the next pair's matmuls instead of stalling the psum ring.
  - proj y^T = wproj^T @ attn at the end; y shipped fp16, host casts.
All PSUM flows through tags pss (2x2 banks) + po (2x2 banks) = 8 banks;
QK/V/pinv tiles ride the pss ring.
"""

import numpy as np

_CACHE = {}

B = 8
WS = 32
N = WS * WS            # 1024 tokens
C = 768
NH = 12
HD = 64
P = 128
QC = 2                 # q chunks of 512
QN = N // QC           # 512
KT = N // P            # 8 k tiles
CT = C // P            # 6 contraction tiles
NPAIR = NH // 2        # 6 head pairs
OT_QK = 2 * CT         # 12 output tiles for q,k rows
VC = 2                 # v output chunks of 384
VN = C // VC           # 384


def _build():
    import concourse.bass as bass
    import concourse.bacc as bacc
    import concourse.mybir as mybir
    import concourse.tile as tile

    f32 = mybir.dt.float32
    f32r = mybir.dt.float32r
    f16 = mybir.dt.float16
    AF = mybir.ActivationFunctionType

    nc = bacc.Bacc(None, target_bir_lowering=False)

    xT_d = nc.dram_tensor("xT", [C, N], f16, kind="ExternalInput")
    # per-pair slabs: [jp][c][0:128]=wq tile (scaled), [128:256]=wk tile
    wqk_d = nc.dram_tensor("wqk", [NPAIR, C, 2 * P], f16, kind="ExternalInput")
    wv_d = nc.dram_tensor("wv", [C, C], f16, kind="ExternalInput")
    wproj_d = nc.dram_tensor("wproj", [C, C], f16, kind="ExternalInput")
    cn_d = nc.dram_tensor("cn", [P, 512], f32, kind="ExternalInput")
    vb_d = nc.dram_tensor("vb", [C], f32, kind="ExternalInput")
    expBT_d = nc.dram_tensor("expBT", [NH, KT, P, N], f16, kind="ExternalInput")
    yT_d = nc.dram_tensor("yT", [C, N], f16, kind="ExternalOutput")

    with tile.TileContext(nc) as tc:
        with (
            tc.tile_pool(name="cst", bufs=1) as cst,
            tc.tile_pool(name="bias_pool", bufs=4) as bias_pool,
            tc.tile_pool(name="pt_pool", bufs=4) as pt_pool,
        ):
            # ---- persistent SBUF ----
            xT_s = cst.tile([P, CT, N], f16)
            wqk_s = cst.tile([P, NPAIR, CT, 2 * P], f16)
            wv_s = cst.tile([P, CT, C], f16)
            wproj_s = cst.tile([P, CT, C], f16)
            cn_s = cst.tile([P, 512], f32)
            q_t = cst.tile([P, NPAIR, N], f16)
            k_t = cst.tile([P, NPAIR, N], f16)
            v_aug = cst.tile([P, KT, NH, HD + 1], f16)
            attn = cst.tile([P, NPAIR, N], f16)
            vb_bc = cst.tile([P, C], f32)
            # selector ones-rows at partitions 0 and 32 (engine accesses must
            # start at 32-aligned partitions, so the two 1/sums rows live at
            # rows 0 and 32 of 33-row tiles)
            sel_s = cst.tile([33, P], f16)
            srows = cst.tile([33, N], f32)
            srinv = cst.tile([33, N], f32)
            s2f16 = cst.tile([33, N], f16)
            warm = cst.tile([1, 16], f32)

            biasb = {}

            def load_bias(h):
                biasb[h] = bias_pool.tile([P, KT, N], f16, tag="biasT",
                                          name=f"biasT{h}")
                nc.sync.dma_start(
                    biasb[h],
                    expBT_d[h].rearrange("kt p q -> p kt q"))

            # ---- DMAs: phase-1-critical first; bias after (it otherwise
            # starves the startup-critical transfers for bandwidth) ----
            xT_src = xT_d[:].rearrange("(k p) t -> p k t", p=P)
            wv_src = wv_d[:].rearrange("(k p) o -> p k o", p=P)
            wproj_src = wproj_d[:].rearrange("(k p) o -> p k o", p=P)
            wqk_src = wqk_d[:].rearrange("jp (k p) o -> p jp k o", p=P)

            nc.sync.dma_start(xT_s[:, 0:3, :], xT_src[:, 0:3, :])
            nc.sync.dma_start(xT_s[:, 3:6, :], xT_src[:, 3:6, :])
            nc.sync.dma_start(wqk_s[:, 0, :, :], wqk_src[:, 0, :, :])
            nc.sync.dma_start(wv_s, wv_src)
            nc.sync.dma_start(cn_s, cn_d[:])
            nc.sync.dma_start(
                vb_bc, bass.AP(tensor=vb_d, offset=0, ap=[[0, P], [1, C]]))
            load_bias(0)
            load_bias(1)
            nc.sync.dma_start(wqk_s[:, 1:NPAIR, :, :],
                              wqk_src[:, 1:NPAIR, :, :])
            qkb_s = cn_s[:, 0:OT_QK]
            pb_s = cn_s[:, 16:16 + CT]

            nc.vector.memset(v_aug[:, :, :, HD:HD + 1], 1.0)
            nc.vector.memset(sel_s[0:1, 0:64], 1.0)
            nc.vector.memset(sel_s[32:33, 0:64], 1.0)
            nc.vector.memset(srinv, 1.0)
            nc.vector.memset(srows, 1.0)
            nc.vector.memset(warm, 0.0)
            # pull the exp table set in during the initial DMA window
            nc.scalar.activation(warm, warm, AF.Exp, bias=0.0, scale=1.0)

            # ---- main PSUM pool: everything rides two tag rings ----
            with tc.tile_pool(name="ps", bufs=1, space="PSUM") as ps:

                def qk_tile(jp, kside):
                    """Produce q_t/k_t tile jp: [128 o, 1024 t]."""
                    o0 = P if kside else 0
                    jj = (CT + jp) if kside else jp
                    for qc in range(QC):
                        pq = ps.tile([P, QN], f32, tag="pss", bufs=4,
                                     name=f"pq{jp}_{kside}_{qc}")
                        for k in range(CT):
                            nc.tensor.matmul(
                                pq,
                                wqk_s[:, jp, k, o0:o0 + P],
                                xT_s[:, k, qc * QN:(qc + 1) * QN],
                                start=(k == 0), stop=(k == CT - 1))
                        dst = k_t if kside else q_t
                        nc.vector.tensor_scalar_add(
                            dst[:, jp, qc * QN:(qc + 1) * QN], pq,
                            qkb_s[:, jj:jj + 1])

                def v_tile(tt):
                    """Produce v_aug rows for t-tile tt (all 12 heads)."""
                    for vc in range(VC):
                        pv = ps.tile([P, QN], f32, tag="pss", bufs=4,
                                     name=f"pv{tt}_{vc}")
                        for k in range(CT):
                            nc.tensor.matmul(
                                pv[:, 0:VN],
                                xT_s[:, k, tt * P:(tt + 1) * P],
                                wv_s[:, k, vc * VN:(vc + 1) * VN],
                                start=(k == 0), stop=(k == CT - 1))
                        h0 = vc * (NH // VC)
                        nc.vector.tensor_add(
                            v_aug[:, tt, h0:h0 + NH // VC, 0:HD],
                            pv[:, 0:VN],
                            vb_bc[:, vc * VN:(vc + 1) * VN])

                pos = {}

                def norm_srows(jp):
                    """copy the pair's sums rows out of psum (ACT)."""
                    po_a, po_b = pos[jp]
                    nc.scalar.activation(srows[0:1, :], po_a[64:65, :],
                                         AF.Identity, bias=0.0, scale=1.0)
                    nc.scalar.activation(srows[32:33, :], po_b[64:65, :],
                                         AF.Identity, bias=0.0, scale=1.0)

                def norm_pair(jp):
                    """1/sums -> broadcast -> evict + apply pair jp."""
                    po_a, po_b = pos.pop(jp)
                    nc.vector.reciprocal_approx_fast(srinv, srows)
                    with nc.allow_low_precision(reason="1/sums in f16"):
                        nc.vector.tensor_copy(s2f16, srinv)
                    nc.vector.tensor_copy(attn[0:64, jp, :], po_a[0:64, :])
                    nc.vector.tensor_copy(attn[64:128, jp, :], po_b[0:64, :])
                    for qc in range(QC):
                        qs = slice(qc * QN, (qc + 1) * QN)
                        pinv = ps.tile([P, QN], f32, tag="pss", bufs=4,
                                       name=f"pinv{jp}_{qc}")
                        nc.tensor.matmul(pinv[0:64, :],
                                         sel_s[0:1, 0:64],
                                         s2f16[0:1, qs],
                                         start=True, stop=True)
                        nc.tensor.matmul(pinv[64:128, :],
                                         sel_s[32:33, 0:64],
                                         s2f16[32:33, qs],
                                         start=True, stop=True)
                        nc.vector.tensor_mul(attn[:, jp, qs],
                                             attn[:, jp, qs], pinv)

                # phase 1 pre: QK pair 0, then V
                qk_tile(0, 0)
                qk_tile(0, 1)
                for tt in range(KT):
                    v_tile(tt)

                # ---- head-pair loop ----
                for jp in range(NPAIR):
                    ha, hb = 2 * jp, 2 * jp + 1
                    if jp > 0:
                        norm_srows(jp - 1)
                    if jp + 1 < NPAIR:
                        qk_tile(jp + 1, 0)
                        qk_tile(jp + 1, 1)
                    if 2 * jp + 2 < NH:
                        load_bias(2 * jp + 2)
                    if jp == 2:
                        nc.sync.dma_start(wproj_s, wproj_src)

                    po_a = ps.tile([HD + 1, N], f32, tag="po", bufs=2,
                                   name=f"poa{jp}")
                    po_b = ps.tile([HD + 1, N], f32, tag="po", bufs=2,
                                   name=f"pob{jp}")
                    pos[jp] = (po_a, po_b)
                    pt2 = {}

                    def s_block(kt, jp=jp, ha=ha, hb=hb, pt2=pt2):
                        g, r = kt // 2, kt % 2
                        for (h, base) in ((ha, 0), (hb, 64)):
                            if r == 0:
                                pt2[(h, g)] = pt_pool.tile(
                                    [P, 2, N], f16, tag="pt",
                                    name=f"pt{h}_{g}")
                            pt = pt2[(h, g)]
                            for qc in range(QC):
                                pS = ps.tile([P, QN], f32, tag="pss", bufs=4,
                                             name=f"pS{h}_{kt}_{qc}")
                                nc.tensor.matmul(
                                    pS,
                                    k_t[base:base + HD, jp,
                                        kt * P:(kt + 1) * P],
                                    q_t[base:base + HD, jp,
                                        qc * QN:(qc + 1) * QN],
                                    start=True, stop=True)
                                nc.scalar.activation(
                                    pt[:, r, qc * QN:(qc + 1) * QN], pS,
                                    AF.Exp, bias=0.0, scale=1.0)
                            if r == 1:
                                nc.vector.tensor_mul(
                                    pt, pt, biasb[h][:, 2 * g:2 * g + 2, :])

                    def av_block(kt, jp=jp, ha=ha, hb=hb, pt2=pt2,
                                 po_a=po_a, po_b=po_b):
                        g, r = kt // 2, kt % 2
                        for (h, po) in ((ha, po_a), (hb, po_b)):
                            pt = pt2[(h, g)]
                            for qc in range(QC):
                                nc.tensor.matmul(
                                    po[:, qc * QN:(qc + 1) * QN],
                                    v_aug[:, kt, h, :],
                                    pt[:, r, qc * QN:(qc + 1) * QN],
                                    start=(kt == 0), stop=(kt == KT - 1))

                    s_block(0)
                    s_block(1)
                    if jp > 0:
                        norm_pair(jp - 1)
                    av_block(0)
                    for kt in range(2, KT):
                        if kt == 4 and 2 * jp + 3 < NH:
                            load_bias(2 * jp + 3)
                        s_block(kt)
                        av_block(kt - 1)
                    av_block(KT - 1)

                norm_srows(NPAIR - 1)
                norm_pair(NPAIR - 1)

            # ---- phase 3: y^T = wproj^T @ attn ----
            with (
                tc.tile_pool(name="p3", bufs=1) as p3,
                tc.tile_pool(name="ps_y", bufs=1, space="PSUM") as ps_y,
            ):
                yT_dst = yT_d[:].rearrange("(j p) t -> p j t", p=P)
                for j in range(CT):
                    py = ps_y.tile([P, N], f32, tag="py", bufs=3,
                                   name=f"py{j}")
                    for k in range(CT):
                        for qc in range(QC):
                            nc.tensor.matmul(
                                py[:, qc * QN:(qc + 1) * QN],
                                wproj_s[:, k, j * P:(j + 1) * P],
                                attn[:, k, qc * QN:(qc + 1) * QN],
                                start=(k == 0), stop=(k == CT - 1))
                    yb = p3.tile([P, N], f16, tag="yb", bufs=2, name=f"yb{j}")
                    nc.vector.tensor_scalar_add(yb, py, pb_s[:, j:j + 1])
                    nc.sync.dma_start(yT_dst[:, j, :], yb)

    nc.compile()
    return nc


def _get_nc():
    if "nc" not in _CACHE:
        _CACHE["nc"] = _build()
    return _CACHE["nc"]


def prepare_inputs(x, qkv_w, q_bias, v_bias, proj_w, proj_b, rel_table,
                   rel_index):
    """Host-side resharding/layout prep. Returns per-core input maps."""
    scale = HD ** -0.5
    x = np.asarray(x, np.float32)
    qkv_w = np.asarray(qkv_w, np.float32)
    q_bias = np.asarray(q_bias, np.float32)
    v_bias = np.asarray(v_bias, np.float32)
    proj_w = np.asarray(proj_w, np.float32)
    proj_b = np.asarray(proj_b, np.float32)
    rel_table = np.asarray(rel_table, np.float32)
    rel_index = np.asarray(rel_index)

    wqT = (qkv_w[0:C, :] * scale).T        # [c, o]
    wkT = qkv_w[C:2 * C, :].T
    wv = qkv_w[2 * C:3 * C, :]
    # per-pair slabs [jp, c, 256]: q j-tile then k j-tile
    wqk = np.empty((NPAIR, C, 2 * P), np.float16)
    for jp in range(NPAIR):
        wqk[jp, :, 0:P] = wqT[:, jp * P:(jp + 1) * P]
        wqk[jp, :, P:2 * P] = wkT[:, jp * P:(jp + 1) * P]
    wv_t = np.ascontiguousarray(wv.T.astype(np.float16))         # [c, C]
    wproj = np.ascontiguousarray(proj_w.T.astype(np.float16))    # [c, co]
    qkb = np.concatenate([q_bias * scale, np.zeros(C, np.float32)])
    qkb = qkb.reshape(OT_QK, P).T                                # [P, 12]
    pb = proj_b.reshape(CT, P).T                                 # [P, 6]
    cn = np.zeros((P, 512), np.float32)
    cn[:, 0:OT_QK] = qkb
    cn[:, 16:16 + CT] = pb

    # bias[q, k, h] = rel_table[rel_index[q, k]]; ship exp(biasT[h, k, q])
    # so the kernel folds the softmax bias multiplicatively into P^T
    bias = rel_table[rel_index.reshape(-1)].reshape(N, N, NH)
    expBT = np.exp(bias.transpose(2, 1, 0), dtype=np.float32)
    expBT = np.ascontiguousarray(
        expBT.reshape(NH, KT, P, N).astype(np.float16))

    shared = {
        "wqk": wqk, "wv": wv_t, "wproj": wproj, "cn": cn,
        "vb": v_bias, "expBT": expBT,
    }
    in_maps = []
    for b in range(B):
        xt = np.ascontiguousarray(
            x[b].reshape(N, C).T.astype(np.float16))
        in_maps.append({"xT": xt, **shared})
    return in_maps


def kernel(x, qkv_w, q_bias, v_bias, proj_w, proj_b, rel_table, rel_index,
           _trace=False):
    from concourse.bass_utils import run_bass_kernel_spmd

    nc = _get_nc()
    in_maps = prepare_inputs(x, qkv_w, q_bias, v_bias, proj_w, proj_b,
                             rel_table, rel_index)
    kwargs = {}
    if _trace:
        import concourse.bass_utils as _bu
        _bu.upload_artifacts = lambda tmpdir: tmpdir
        kwargs = {"trace": True}
    res = run_bass_kernel_spmd(nc, in_maps, core_ids=list(range(B)), **kwargs)
    out = np.empty((B, WS, WS, C), np.float32)
    for b in range(B):
        out[b] = res.results[b]["yT"].astype(np.float32).T.reshape(WS, WS, C)
    if _trace:
        _CACHE["last_result"] = res
    return out
